# revision 25
# baseline (speedup 1.0000x reference)
"""Trainium2 Bass kernel for nn_Network_81862076662591 (sampling network).

Self-contained: takes FULL inputs (as produced by the problem's
setup_inputs), data-parallel shards batch B=256 over 8 NeuronCores
(32 rows each, per-iteration weights replicated), runs the fused
LSTM + gumbel-argmax sampling + MLP scan on-device, and returns the
full [256, 10, 100] output.

Design notes (per core, B=32, T=100, D=784, G=256, H=128):
  - all weights are bf16 on device: fp32 matmuls lower to 2x (HI/LO)
    LDWEIGHTS+MATMUL pairs on trn2, bf16 is single-pass and gets
    fast-weight-load on full 128-col tiles. Verified numerically
    (argmax decisions unchanged, rel err ~6e-3 vs the fp32 reference).
  - per-iteration weights are packed host-side into two HBM blobs so
    each iteration issues 3 large contiguous DMAs (triple-buffered)
    instead of 8 strided ones. W1/W2 (contraction D=784) are
    zero-padded to 896 = 7*128 rows so every lhsT tile is [128, 128].
  - everything stays feature-major [feat, batch]; all MLP matmuls are
    weight-stationary with the tiny [*, 32] activations streaming.
  - sampling feature-major: pert^T = logits^T + (gum^T - 1e9*mem^T);
    2-stage argmax (DVE k-reduce -> PE transpose -> DVE row max), then
    one-hot = (pert == k-max) * winning-row-indicator via is_equal
    (no ties on this data, verified); st = hard + soft -
    stop_grad(soft) == hard numerically, so softmax is skipped.
  - the gumbel tensor is pre-transposed host-side with -1e9 in the
    896-pad rows so padded features never win the argmax; bg2 is
    folded into it (all other biases are zeros, asserted).
  - PSUM start=True clears has_written for the whole bank, so split
    accumulation groups (LSTM gates prefetch the h@Whh half an
    iteration early) put start=True only on their first matmul.
  - persistent PSUM tiles with once-zeroed dead regions (ps_lf, ps_a1,
    ps_tr) let each consumer be one full-size DVE/ACT op.
  - leaky relu (slope 0.2) = ScalarE Prelu (NOT Lrelu, whose alpha is
    ignored); sigmoid/tanh/parametric_relu share one ACT table set.
    Gate columns are permuted host-side to [i,f,o,g] so the three
    sigmoids are one activation call.
  - tiny keep-warm matmuls chained to serial stages feed the PE HAM
    activity monitor through activation-heavy windows.
"""
from contextlib import ExitStack

import numpy as np
import ml_dtypes

import concourse.bass as bass
import concourse.mybir as mybir
import concourse.tile as tile
from concourse.vector_clock import ScopedClock
from concourse.bass_utils import run_bass_kernel_spmd

F32 = mybir.dt.float32
BF16 = mybir.dt.bfloat16
ALU = mybir.AluOpType
ACTF = mybir.ActivationFunctionType

NCORES = 8
B = 32          # per-core batch
D = 784
DP = 896        # D zero-padded to 7*128 for full-width lhsT tiles
KC = 128        # k-chunk width over the (padded) D axis
TW = 16         # valid width of the 7th (tail) chunk: 784 - 6*128
NK = 7
G = 256
H = 128
T = 100
NEGBIG = -1.0e9

# blobA per-partition column layout (bf16), 128 partitions:
#   W1 section: 7 k-chunks x 784   (W1pad[k*128+p, n])
#   W2 section: 7 k-chunks x 256   (W2pad[k*128+p, g])
A_W1 = 0
A_W2 = NK * D                 # 5488
A_COLS = NK * D + NK * G      # 7280

# blobB per-partition column layout (bf16), 128 partitions:
B_WG2 = 0                     # 2 k-chunks x 784 (Wg2[k*128+p, n])
B_WG1 = 2 * D                 # 256  (Wg1[p, g])
B_W3 = B_WG1 + G              # 2 k-chunks x 128 (W3[k*128+p, j])
B_W4 = B_W3 + 2 * H           # 128
B_WF2 = B_W4 + H              # 10
B_COLS = B_WF2 + 10           # 2218

USE_LRELU_ACT = True          # leaky relu on ScalarE; False -> DVE stt


class _TileContextSplitDrain(tile.TileContext):
    """This walrus build rejects >1 sem-wait on the kernel-tail Drain;
    split the accumulated waits across several sequential drains."""

    def _drain_and_barrier(self, tick_clock, wait_clock):
        drain_inst = self.nc.sync.drain()
        wait_clock.add_sem_waits(
            drain_inst.ins, ScopedClock({None: tick_clock.global_clock}))
        si = drain_inst.ins.sync_info
        waits = list(si.on_wait or []) if si is not None else []
        if len(waits) > 1:
            si.on_wait = [waits[0]]
            for w in waits[1:]:
                d2 = self.nc.sync.drain()
                if d2.ins.sync_info is None:
                    d2.ins.sync_info = mybir.SyncInfo(on_wait=[w], on_update=[])
                else:
                    d2.ins.sync_info.on_wait = [w]
        self.nc.all_engine_barrier()
        assert self.sems is not None
        popped = self.nc._tile_sem_poison_stack.pop()
        assert popped is self._sem_poison
        self.nc.clear_and_free_semaphores(list(self.sems.allocated().values()))
        self.nc.all_engine_barrier()


def _split_multi_waits(nc, limit=1):
    """This walrus accepts only `limit` sem-waits per instruction; move the
    excess onto same-engine sequencer NOPs inserted immediately before."""
    import copy

    proto = nc.vector.isa(nc.isa.Opcode.NEURON_ISA_TPB_OPCODE_NOP, {}).ins
    nop_ctr = [0]

    def make_nop(engine, waits):
        nop = copy.deepcopy(proto)
        nop_ctr[0] += 1
        nop.name = f"waitnop-{nop_ctr[0]}"
        nop.engine = engine
        nop.sync_info = mybir.SyncInfo(on_wait=list(waits), on_update=[])
        return nop

    skip = ("InstAllEngineBarrier", "InstEventSemaphore")
    for fn in nc.m.functions:
        for bb in fn.blocks:
            insts = bb.instructions
            if insts and insts[-1] is proto:
                insts.pop()
            out = []
            for inst in insts:
                si = inst.sync_info
                waits = list(si.on_wait or []) if si is not None else []
                if len(waits) > limit and type(inst).__name__ not in skip:
                    for i in range(0, len(waits) - limit, limit):
                        out.append(make_nop(inst.engine, waits[i:i + limit]))
                    si.on_wait = waits[len(waits) - limit:]
                out.append(inst)
            bb.instructions[:] = out


def _declare_params(nc):
    dp = nc.declare_dram_parameter
    p = {}
    p["x"] = dp("x", [B, D], F32, isOutput=False)
    # gumbel noise pre-transposed feature-major [T, KC, NK, B]; the dead
    # pad region [TW:, 6, :] is filled with -1e9 host-side
    p["gum"] = dp("gum", [T, KC, NK * B], F32, isOutput=False)
    p["blobA"] = dp("blobA", [T, 128, A_COLS], BF16, isOutput=False)
    p["blobB"] = dp("blobB", [T, 128, B_COLS], BF16, isOutput=False)
    p["Wih"] = dp("Wih", [H, 4 * H], BF16, isOutput=False)
    p["Whh"] = dp("Whh", [H, 4 * H], BF16, isOutput=False)
    p["ident"] = dp("ident", [B, B], F32, isOutput=False)
    p["ident128"] = dp("ident128", [128, 128], F32, isOutput=False)
    p["out"] = dp("out", [10, T * B], F32, isOutput=True)
    return p


def _leaky(nc, out_ap, in_ap, tmp_ap=None):
    """out = leaky_relu(in_, slope 0.2) on ScalarE."""
    nc.scalar.activation(out_ap, in_ap, ACTF.Prelu, alpha=0.2)


def _leaky_v(nc, out_ap, in_ap):
    """Leaky relu for split consumers (walrus rejects the DVE stt form
    with PSUM operands, so this also lowers to the ScalarE Prelu)."""
    nc.scalar.activation(out_ap, in_ap, ACTF.Prelu, alpha=0.2)


def _build(ctx, tc, p, w_bufs=3):
    nc = tc.nc

    const_pool = ctx.enter_context(tc.tile_pool(name="const", bufs=1))
    state_pool = ctx.enter_context(tc.tile_pool(name="state", bufs=1))
    wpool = ctx.enter_context(tc.tile_pool(name="w", bufs=w_bufs))
    psum = ctx.enter_context(tc.tile_pool(name="ps", bufs=1, space="PSUM"))

    WihS = const_pool.tile([H, 4 * H], BF16, tag="wih")
    WhhS = const_pool.tile([H, 4 * H], BF16, tag="whh")
    IDENT = const_pool.tile([B, B], F32, tag="ident")
    IDENT128 = const_pool.tile([128, 128], F32, tag="id128")
    XB = const_pool.tile([B, D], F32, tag="xb")
    nc.sync.dma_start(WihS[:], p["Wih"].ap())
    nc.sync.dma_start(WhhS[:], p["Whh"].ap())
    nc.sync.dma_start(IDENT[:], p["ident"].ap())
    nc.sync.dma_start(IDENT128[:], p["ident128"].ap())
    nc.sync.dma_start(XB[:], p["x"].ap())

    A_fm = state_pool.tile([H, B], BF16, tag="a")        # lin^T (bf16)
    H_fm = state_pool.tile([H, B], BF16, tag="h")
    # TGC packs [tanh(g); c] so the two LSTM cell products are one DVE op
    TGC = state_pool.tile([H, 2, B], F32, tag="tgc")
    U01 = state_pool.tile([H, 2, B], F32, tag="u01")
    # x^T bf16: y is bf16 anyway, and 16-bit inputs put the one-hot*x
    # multiply in the DVE 2x mode
    XFM = state_pool.tile([KC, NK, B], BF16, tag="xfm")
    YT = state_pool.tile([KC, NK, B], BF16, tag="yt")    # (mem*x)^T bf16
    MEMFM = state_pool.tile([KC, NK, B], F32, tag="memfm")  # mask^T
    TMX = state_pool.tile([KC, B], F32, tag="tmx")       # max over k
    MAXV = state_pool.tile([B, 1], F32, tag="maxv")      # global max
    E1 = state_pool.tile([B, KC], F32, tag="e1")         # argmax chunk-row
    EQ2 = state_pool.tile([KC, NK, B], F32, tag="eq2")
    P1 = state_pool.tile([KC, NK, B], BF16, tag="p1")    # EQ2 * x^T
    HDF = state_pool.tile([KC, NK, B], BF16, tag="hdf")  # one-hot fm (exact)
    HXF = state_pool.tile([KC, NK, B], BF16, tag="hxf")  # one-hot * x fm
    SIFO = state_pool.tile([H, 4, B], F32, tag="sifo")   # sig(i,f,o,2g)
    TC = state_pool.tile([H, B], F32, tag="tc")
    G1 = state_pool.tile([H, 2, B], BF16, tag="g1")
    A1 = state_pool.tile([128, NK, B], BF16, tag="a1")   # f1 layer1, m-tiled
    A2 = state_pool.tile([H, 2, B], BF16, tag="a2")
    A3 = state_pool.tile([H, B], BF16, tag="a3")
    SOUT = state_pool.tile([10, T, B], F32, tag="sout")

    nc.vector.memset(A_fm[:], 0.0)
    nc.vector.memset(H_fm[:], 0.0)
    nc.vector.memset(TGC[:], 0.0)
    nc.vector.memset(YT[:], 0.0)
    nc.vector.memset(MEMFM[:], 0.0)
    nc.vector.memset(XFM[:], 0.0)
    # A1/YT/XFM chunk 6 is only 16 partitions valid; zero the dead regions
    # once so the zero-padded W1/W2 k-chunk 6 contracts against zeros.
    nc.vector.memset(A1[:], 0.0)

    # x^T feature-major (7 PE transposes of [B,128] chunks, 16-wide tail).
    # ps_tr persists across iterations: its dead region [TW:, 6, :] is
    # zeroed once so the per-iteration y^T update is a single DVE op.
    # Cols 7:11 host the TMX-transpose dest (the old ps_warm bank is
    # spent on the third f1-layer-1 accumulator instead).
    ps_tr = psum.tile([KC, NK + 4, B], F32, tag="tr")
    nc.vector.memset(ps_tr[:, 6, :], 0.0)
    for k in range(NK):
        kw = KC if k < 6 else TW
        nc.tensor.transpose(ps_tr[0:kw, k, :],
                            XB[:, k * KC:k * KC + kw], IDENT[:])
    nc.scalar.copy(XFM[:, 0:6, :], ps_tr[:, 0:6, :])
    nc.scalar.copy(XFM[0:TW, 6, :], ps_tr[0:TW, 6, :])

    # persistent PSUM tiles: fm logits and f1-layer-1 accumulators (dead
    # tail regions zeroed once so each consumer is a single full-size op)
    # and the scratch bank for the TMX transpose + keep-warm dummies
    ps_lf = psum.tile([KC, NK, B], F32, tag="lf")
    # f1 layer-1 accumulator in THREE banks (m 0..2 / 3..4 / 5..6): the
    # tile framework serializes a bank's matmul writes after any
    # outstanding read of that bank, so each leaky's read must not share
    # a bank with still-streaming later m-tiles.
    ps_a1a = psum.tile([128, 3, B], F32, tag="a1a")
    ps_a1c = psum.tile([128, 2, B], F32, tag="a1c")
    ps_a1b = psum.tile([128, 2, B], F32, tag="a1b")
    nc.vector.memset(ps_a1b[:, 1, :], 0.0)
    # ps_sm is persistent so the classifier region [0:10, 2, :] can carry
    # iteration t-1's result across the loop-top boundary
    ps_sm = psum.tile([H, 5, B], F32, tag="sm")

    # set ps_lf's has_written bits once over the full [128, 224] region
    # (values are garbage). Every later Wg2 matmul uses start=False, so
    # the bits stay set forever and matmuls ACCUMULATE onto the per-
    # iteration DVE-preloaded masked-gumbel values (PSUM-as-bias trick:
    # only TensorE touches has_written; DVE writes leave the bits alone).
    nc.tensor.matmul(ps_lf[:].rearrange("p k b -> p (k b)"),
                     A1[:, 0:4, :].rearrange("p k b -> p (k b)"),
                     YT[:].rearrange("p k b -> p (k b)"),
                     start=True, stop=True)

    # prologue: Whh half of iteration 0's gates (h starts at zero).
    # NOTE start=True clears has_written for the WHOLE bank, so it may
    # appear only on the first matmul of the 8-matmul gate group.
    ps_g = psum.tile([H, 4, B], F32, tag="g")
    for j in range(4):
        nc.tensor.matmul(ps_g[:, j, :], WhhS[:, j * H:(j + 1) * H],
                         H_fm[:], start=(j == 0), stop=False,
                         skip_group_check=True)

    WB_prev = None
    for t in range(T):
        # ---- weight/noise loads for iteration t (3 contiguous DMAs)
        WA = wpool.tile([128, A_COLS], BF16, tag="wa")
        WB = wpool.tile([128, B_COLS], BF16, tag="wb")
        GUM = wpool.tile([KC, NK, B], F32, tag="gum")
        nc.sync.dma_start(WA[:], p["blobA"].ap()[t])
        nc.sync.dma_start(WB[:], p["blobB"].ap()[t])
        nc.sync.dma_start(
            GUM[:], p["gum"].ap()[t].rearrange("p (k b) -> p k b", k=NK))

        # preload masked gumbel noise INTO the logits PSUM bank (off the
        # critical path: DVE slot in the f1/LSTM window). The Wg2 matmuls
        # then accumulate on top (start=False; has_written set at init),
        # so pert == ps_lf with no separate add on the critical path.
        nc.vector.scalar_tensor_tensor(ps_lf[:], MEMFM[:], NEGBIG, GUM[:],
                                       ALU.mult, ALU.add)

        # ---- LSTM cell: finish gates = lin @ Wih + (h @ Whh, prefetched)
        # host permutes gate columns to [i, f, o, g] and doubles the g
        # block so tanh(g) = 2*sigmoid(2g) - 1 comes out of the same
        # 4-gate sigmoid call
        for j in range(4):
            nc.tensor.matmul(ps_g[:, j, :], WihS[:, j * H:(j + 1) * H],
                             A_fm[:], start=False, stop=(j == 3),
                             skip_group_check=True)
        # classifier for iteration t-1 (off-path: issued after the gate
        # matmuls so it never delays them)
        if WB_prev is not None:
            nc.tensor.matmul(ps_sm[0:10, 2, :],
                             WB_prev[:, B_WF2:B_WF2 + 10], A_fm[:],
                             start=True, stop=True)
            nc.vector.tensor_copy(SOUT[:, t - 1, :], ps_sm[0:10, 2, :])
        WB_prev = WB

        # f1 layer-1 EARLY half: y(t-1) @ W1[t] streams through the
        # otherwise idle PE during the LSTM/gating/sampling window (it
        # only needs WA[t] and last iteration's y). The late half adds
        # the delta (one-hot*x) after the argmax; PSUM accumulates both.
        def _w1_half(rhs, first, stop):
            for bank, ms in ((ps_a1a, (0, 1, 2)), (ps_a1c, (3, 4)),
                             (ps_a1b, (5, 6))):
                for mi, m in enumerate(ms):
                    mw = 128 if m < 6 else TW
                    for k in range(NK):
                        nc.tensor.matmul(
                            bank[0:mw, mi, :],
                            WA[:, A_W1 + k * D + m * 128:
                               A_W1 + k * D + m * 128 + mw],
                            rhs[:, k, :],
                            start=(first and mi == 0 and k == 0),
                            stop=(stop and k == NK - 1),
                            skip_group_check=True)

        _w1_half(YT, first=True, stop=False)

        nc.scalar.activation(SIFO[:], ps_g[:], ACTF.Sigmoid)
        # tanh(g) = 2*sigmoid(2g) - 1 on the DVE (g pre-doubled in W)
        nc.vector.tensor_scalar(TGC[:, 0, :], SIFO[:, 3, :], 2.0, -1.0,
                                ALU.mult, ALU.add)
        # c = sig_f*c + sig_i*tanh(g): both products in one DVE op via the
        # [tanh(g); c] packing, then one add
        nc.vector.tensor_tensor(U01[:], SIFO[:, 0:2, :], TGC[:], ALU.mult)
        nc.vector.tensor_tensor(TGC[:, 1, :], U01[:, 0, :], U01[:, 1, :],
                                ALU.add)
        nc.scalar.activation(TC[:], TGC[:, 1, :], ACTF.Tanh)
        nc.vector.tensor_tensor(H_fm[:], SIFO[:, 2, :], TC[:], ALU.mult)

        # ---- gating MLP first (on the critical path); the Whh prefetch
        # of the next iteration's gates is issued after the G1 leaky so
        # the lowered PE program keeps Wg1 in front
        for m in range(2):
            nc.tensor.matmul(ps_sm[:, 3 + m, :],
                             WB[:, B_WG1 + m * H:B_WG1 + (m + 1) * H],
                             H_fm[:], start=True, stop=True)
        _leaky(nc, G1[:], ps_sm[:, 3:5, :])
        ps_g = psum.tile([H, 4, B], F32, tag="g")
        for j in range(4):
            nc.tensor.matmul(ps_g[:, j, :], WhhS[:, j * H:(j + 1) * H],
                             H_fm[:], start=(j == 0), stop=False,
                             skip_group_check=True)
        # logits accumulate ONTO the preloaded masked gumbel (start=False
        # always; ps_lf == pert after the last stop). The 16-wide m=6
        # tail goes first within each k so its slower weight load hides
        # under the previous group's stream.
        for k in range(2):
            for m in (6, 0, 1, 2, 3, 4, 5):
                mw = 128 if m < 6 else TW
                nc.tensor.matmul(ps_lf[0:mw, m, :],
                                 WB[:, k * D + m * 128:k * D + m * 128 + mw],
                                 G1[:, k, :],
                                 start=False,
                                 stop=(k == 1 and m == 5),
                                 skip_group_check=True)

        # ---- sampling (feature-major): 2-stage max over pert == ps_lf
        TRT = ps_tr[0:B, 7:11, :].rearrange("p k b -> p (k b)")
        nc.vector.tensor_reduce(TMX[:], ps_lf[:].rearrange("p k b -> p b k"),
                                axis=mybir.AxisListType.X, op=ALU.max)
        nc.tensor.transpose(TRT, TMX[:], IDENT128[:])
        # EQ2 (per-chunk argmax indicator) overlaps the TMX transpose
        nc.vector.tensor_tensor(
            EQ2[:], ps_lf[:],
            TMX[:].unsqueeze(1).broadcast_to((KC, NK, B)), ALU.is_equal)
        nc.vector.tensor_reduce(MAXV[:], TRT,
                                axis=mybir.AxisListType.X, op=ALU.max)
        # E1 as tensor_tensor (broadcast MAXV along the free dim): the
        # reduce->TT handoff is ~200ns cheaper than reduce->tensor_scalar
        nc.vector.tensor_tensor(E1[:], TRT,
                                MAXV[:].broadcast_to((B, KC)), ALU.is_equal)
        nc.tensor.transpose(ps_tr[:, 0, :], E1[:], IDENT[:])
        # P1 = EQ2 * x^T overlaps the E1 transpose, so only HXF remains
        # on the path between the argmax and the late W1 half
        nc.vector.tensor_tensor(P1[:], EQ2[:], XFM[:], ALU.mult)
        nc.vector.tensor_tensor(
            HXF[:], P1[:],
            ps_tr[:, 0, :].unsqueeze(1).broadcast_to((KC, NK, B)), ALU.mult)

        # ---- f1 MLP layer 1, LATE half: accumulate (one-hot*x) @ W1 on
        # top of the early y(t-1) half. m-tiles 0..2 / 3..4 / 5..6 in
        # separate banks so each leaky runs while later tiles stream.
        def _w1_late_bank(bank, ms):
            for mi, m in enumerate(ms):
                mw = 128 if m < 6 else TW
                for k in range(NK):
                    nc.tensor.matmul(
                        bank[0:mw, mi, :],
                        WA[:, A_W1 + k * D + m * 128:
                           A_W1 + k * D + m * 128 + mw],
                        HXF[:, k, :],
                        start=False, stop=(k == NK - 1),
                        skip_group_check=True)

        _w1_late_bank(ps_a1a, (0, 1, 2))
        _leaky_v(nc, A1[:, 0:3, :], ps_a1a[:])
        _w1_late_bank(ps_a1c, (3, 4))
        _leaky_v(nc, A1[:, 3:5, :], ps_a1c[:])
        _w1_late_bank(ps_a1b, (5, 6))
        _leaky_v(nc, A1[:, 5:7, :], ps_a1b[:])

        # y / one-hot / mask updates for the next iteration (off-path:
        # DVE slots fall in the f1-MLP window)
        nc.vector.tensor_tensor(YT[:], HXF[:], YT[:], ALU.add)
        nc.vector.tensor_tensor(
            HDF[:], EQ2[:],
            ps_tr[:, 0, :].unsqueeze(1).broadcast_to((KC, NK, B)), ALU.mult)
        nc.vector.tensor_tensor(MEMFM[:], HDF[:], MEMFM[:], ALU.add)

        # ---- layer 2: a2 = leaky(a1 @ W2), k-chunks follow A1 m-tiling.
        # k 0..4 go right after leakyA; the paused groups resume with
        # k 5..6 once the A1 tail is through its leaky.
        # NOTE start=True only on the bank's FIRST matmul: a later
        # start=True would clear the whole bank's has_written bits and
        # the paused m=0 group would lose its k 0..4 partial sums.
        # (m=1's k=0 write lands on cleared bits, which overwrites.)
        ps_a2 = psum.tile([H, 2, B], F32, tag="a2")
        for ks in ((0, 1, 2), (3, 4), (5, 6)):
            for m in range(2):
                for k in ks:
                    nc.tensor.matmul(ps_a2[:, m, :],
                                     WA[:, A_W2 + k * G + m * H:
                                        A_W2 + k * G + (m + 1) * H],
                                     A1[:, k, :],
                                     start=(m == 0 and k == 0),
                                     stop=(k == NK - 1),
                                     skip_group_check=True)
        _leaky(nc, A2[:], ps_a2[:])

        # ---- layers 3/4 (reuses ps_sm banks 0..2)
        for k in range(2):
            nc.tensor.matmul(ps_sm[:, 0, :],
                             WB[:, B_W3 + k * H:B_W3 + (k + 1) * H],
                             A2[:, k, :], start=(k == 0), stop=(k == 1))
        _leaky(nc, A3[:], ps_sm[:, 0, :])
        nc.tensor.matmul(ps_sm[:, 1, :], WB[:, B_W4:B_W4 + H], A3[:],
                         start=True, stop=True)
        _leaky(nc, A_fm[:], ps_sm[:, 1, :])
        # classifier f2[t] + SOUT copy happen at the top of iteration t+1
        # (off the critical path); the final iteration's flush is below.

    nc.tensor.matmul(ps_sm[0:10, 2, :], WB_prev[:, B_WF2:B_WF2 + 10],
                     A_fm[:], start=True, stop=True)
    nc.vector.tensor_copy(SOUT[:, T - 1, :], ps_sm[0:10, 2, :])

    nc.sync.dma_start(p["out"].ap(), SOUT[:].rearrange("c t b -> c (t b)"))


_CACHE = {}
TRACE = False
LAST_RES = None


def _get_nc(w_bufs=3):
    key = ("nc", w_bufs)
    if key not in _CACHE:
        nc = bass.Bass("TRN2", target_bir_lowering=False, debug=False)
        p = _declare_params(nc)
        with _TileContextSplitDrain(nc) as tc:
            with ExitStack() as ctx:
                _build(ctx, tc, p, w_bufs=w_bufs)
        _split_multi_waits(nc)
        _CACHE[key] = nc
    return _CACHE[key]


def _pack_blobs(f):
    """Host-side weight packing into the two per-iteration DMA blobs."""
    bf = ml_dtypes.bfloat16
    W1 = f("W1")          # [T, 784, 784]
    W2 = f("W2")          # [T, 784, 256]
    W1p = np.zeros((T, DP, D), np.float32)
    W1p[:, :D] = W1
    W2p = np.zeros((T, DP, G), np.float32)
    W2p[:, :D] = W2
    a1 = W1p.reshape(T, NK, 128, D).transpose(0, 2, 1, 3).reshape(T, 128, NK * D)
    a2 = W2p.reshape(T, NK, 128, G).transpose(0, 2, 1, 3).reshape(T, 128, NK * G)
    blobA = np.ascontiguousarray(
        np.concatenate([a1, a2], axis=2)).astype(bf)

    Wg2 = f("Wg2")        # [T, 256, 784]
    Wg1 = f("Wg1")        # [T, 128, 256]
    W3 = f("W3")          # [T, 256, 128]
    W4 = f("W4")          # [T, 128, 128]
    Wf2 = f("Wf2")        # [T, 128, 10]
    b1 = Wg2.reshape(T, 2, 128, D).transpose(0, 2, 1, 3).reshape(T, 128, 2 * D)
    b3 = W3.reshape(T, 2, 128, H).transpose(0, 2, 1, 3).reshape(T, 128, 2 * H)
    blobB = np.ascontiguousarray(np.concatenate(
        [b1, Wg1, b3, W4, Wf2], axis=2)).astype(bf)
    return blobA, blobB


def kernel(**inputs) -> np.ndarray:
    f = lambda k: np.ascontiguousarray(np.asarray(inputs[k]), dtype=np.float32)
    x = f("x")
    gumbel = f("gumbel")
    bg2 = f("bg2")
    gum_all = gumbel + bg2[:, None, :]          # fold bg2 into the noise
    # remaining biases are zeros in this problem; verify cheaply
    for bn in ("b1", "b2", "b3", "b4", "bf2", "bg1", "bih", "bhh"):
        if bn in inputs and np.any(np.asarray(inputs[bn])):
            raise NotImplementedError(f"nonzero bias {bn} not supported")

    blobA, blobB = _pack_blobs(f)
    bf = ml_dtypes.bfloat16
    # permute gate blocks from torch order [i,f,g,o] to [i,f,o,g] so all
    # four sigmoids are one activation call, and double the g block so
    # tanh(g) = 2*sigmoid(2g) - 1
    perm = np.r_[0:H, H:2 * H, 3 * H:4 * H, 2 * H:3 * H]
    gscale = np.ones((1, 4 * H), np.float32)
    gscale[:, 3 * H:] = 2.0
    shared = {
        "blobA": blobA,
        "blobB": blobB,
        "Wih": np.ascontiguousarray(f("Wih")[:, perm] * gscale).astype(bf),
        "Whh": np.ascontiguousarray(f("Whh")[:, perm] * gscale).astype(bf),
        "ident": np.eye(B, dtype=np.float32),
        "ident128": np.eye(128, dtype=np.float32),
    }

    in_maps = []
    for c in range(NCORES):
        sl = slice(c * B, (c + 1) * B)
        m = dict(shared)
        m["x"] = np.ascontiguousarray(x[sl])
        # gumbel noise feature-major [T, KC, NK, B]; dead pad rows get
        # -1e9 so padded features can never win the argmax
        gc = gum_all[:, sl]                     # [T, B, D]
        gf = np.full((T, KC, NK, B), NEGBIG, np.float32)
        for k in range(NK):
            kw = KC if k < 6 else TW
            gf[:, 0:kw, k, :] = gc[:, :, k * KC:k * KC + kw].transpose(0, 2, 1)
        m["gum"] = gf.reshape(T, KC, NK * B)
        in_maps.append(m)

    nc = _get_nc()
    global LAST_RES
    res = run_bass_kernel_spmd(nc, in_maps, list(range(NCORES)), trace=TRACE)
    LAST_RES = res
    # per-core out is [10, T*B] feature-major; reassemble to [B, 10, T]
    outs = []
    for c in range(NCORES):
        o = res.results[c]["out"].reshape(10, T, B)
        outs.append(np.ascontiguousarray(o.transpose(2, 0, 1)))
    return np.concatenate(outs, axis=0).astype(np.float32)

